# revision 22
# baseline (speedup 1.0000x reference)
# kernel.py — Trainium2 Bass kernel for nn_DenseGridNet (bilinear grid sample + MLP)
#
# Strategy (data-parallel over 8 NeuronCores, sorted sharding):
#  * host: computes cell ids + bilinear coefs (exact fp32 replica of the
#    reference), sorts points by cell, builds an fp16 patch table with
#    256B rows PT[cell] = [v00,v10,v01,v11, pad...] (dma_gather needs
#    256B-aligned elements) and per-chunk 8192-row windows with int16
#    window-local indices.
#  * device, per 8192-point chunk: bulk dma_gather of 256B fp16 patch
#    rows; DVE multiplies the 4 bilinear coefs (host-sent, fp16) into the
#    16 patch values; one xbar DMA transpose flips point-major ->
#    feature-major; TensorE runs the 3 MLP layers as block-diagonal fp16
#    matmuls (2 pts/col); relu is split DVE/ACT; the third layer
#    accumulates all 4 quarters into one [32, 1024] PSUM tile via
#    zero-padded lhsT variants so a single sigmoid serves the chunk;
#    output lands as fp16 yT [32, n_chunks*1024] and is un-permuted on
#    host.
import os
import numpy as np

RX = 1024
RY = 1024
F = 4
HID = 64
N_CORES = 8
P = 128          # partitions
SLOT = 32        # fp16 columns per point in the pre-transpose buffer
WIN = 8192       # dma_gather window rows (int16-addressable)
CHPTS = 8192     # points per gather/transpose chunk
PTROW = 128      # padded patch row, fp16 elements (256B)
QCOLS = 512
XS = 544         # DVE/ACT relu column split (of 1024)


def _build_bass(B):
    """Bass program for one core processing B sorted points."""
    import concourse.bass as bass
    import concourse.tile as tile
    from concourse import bacc, library_config
    import concourse.mybir as mybir

    dt = mybir.dt
    T = B // P
    n_chunks = B // CHPTS
    ch_cols = CHPTS // P            # 64 point cols per chunk
    n_blk = ch_cols * SLOT // 128   # 16 transposed blocks per chunk

    nc = bacc.Bacc(None, target_bir_lowering=False)

    f32 = dt.float32
    f16 = dt.float16

    # ---- DRAM I/O -------------------------------------------------------
    c4_d = nc.dram_tensor("c4pm", [P, T * 4], f16, kind="ExternalInput")
    idf_d = nc.dram_tensor("idfpm", [P, T], f16, kind="ExternalInput")
    ptw_d = nc.dram_tensor("ptw", [n_chunks * WIN, PTROW], f16,
                           kind="ExternalInput")
    idx_d = nc.dram_tensor("idx16", [P, n_chunks * CHPTS // 16], dt.int16,
                           kind="ExternalInput")
    l1_d = nc.dram_tensor("lhsT1", [128, 128], f16, kind="ExternalInput")
    l2_d = nc.dram_tensor("lhsT2", [128, 128], f16, kind="ExternalInput")
    l3_d = nc.dram_tensor("lhsT3", [128, 128], f16, kind="ExternalInput")
    id_d = nc.dram_tensor("ident", [128, 128], f16, kind="ExternalInput")
    b2_d = nc.dram_tensor("b2rep", [128, 1], f32, kind="ExternalInput")
    b3_d = nc.dram_tensor("b3rep", [32, 1], f32, kind="ExternalInput")
    out_d = nc.dram_tensor("yT", [32, n_chunks * 2 * QCOLS], f16,
                           kind="ExternalOutput")

    with tile.TileContext(nc) as tc:
        with (
            tc.tile_pool(name="persist", bufs=1) as pp,
            tc.tile_pool(name="psum_l1", bufs=1, space="PSUM") as ps1,
            tc.tile_pool(name="psum_l2", bufs=1, space="PSUM") as ps2,
            tc.tile_pool(name="psum_l3", bufs=1, space="PSUM") as ps3,
            tc.tile_pool(name="psum_tr", bufs=1, space="PSUM") as pst,
            tc.tile_pool(name="mlp", bufs=4) as mp,
            tc.tile_pool(name="outp", bufs=3) as op_,
        ):
            AL = mybir.AluOpType
            V = nc.vector
            nc.gpsimd.load_library(library_config.mlp)

            c4t = pp.tile([P, T, 4], f16, tag="c4t")
            idft = pp.tile([P, T], f16, tag="idft")
            l1w = pp.tile([128, 128], f16, tag="l1w")
            l2w = pp.tile([128, 128], f16, tag="l2w")
            l3w = pp.tile([128, 4, 32], f16, tag="l3w")
            idw = pp.tile([128, 128], f16, tag="idw")
            b2r = pp.tile([128, 1], f32, tag="b2r")
            b3r = pp.tile([32, 1], f32, tag="b3r")

            NBUF = 2
            idxall = pp.tile([P, n_chunks * CHPTS // 16], dt.int16,
                             tag="idxall")
            patch = [pp.tile([P, ch_cols, PTROW], f16, name=f"patch{i}",
                             tag=f"patch{i}") for i in range(NBUF)]
            qpm = [pp.tile([P, ch_cols, SLOT], f16, name=f"qpm{i}",
                           tag=f"qpm{i}") for i in range(NBUF)]
            qT = [pp.tile([P, n_blk, 128], f16, name=f"qT{i}", tag=f"qT{i}")
                  for i in range(NBUF)]

            # idx first (small piece, then the rest) so chunk-0 desc-gen
            # starts early; remaining loads queue behind the first gathers.
            IW = CHPTS // 16
            nc.sync.dma_start(idxall[:, 0:2 * IW], idx_d[:, 0:2 * IW])
            nc.sync.dma_start(idxall[:, 2 * IW:8 * IW], idx_d[:, 2 * IW:8 * IW])
            nc.sync.dma_start(idxall[:, 8 * IW:], idx_d[:, 8 * IW:])
            nc.sync.dma_start(c4t[:].rearrange("p t c -> p (t c)"), c4_d[:])
            nc.sync.dma_start(idft[:], idf_d[:])
            nc.sync.dma_start(l1w[:], l1_d[:])
            nc.sync.dma_start(l2w[:], l2_d[:])
            nc.sync.dma_start(l3w[:].rearrange("p a b -> p (a b)"), l3_d[:])
            nc.sync.dma_start(idw[:], id_d[:])
            nc.sync.dma_start(b2r[:], b2_d[:])
            nc.sync.dma_start(b3r[:], b3_d[:])

            for i in range(NBUF):
                nc.vector.memset(qpm[i][:, :, 16:17], 1.0)
                nc.vector.memset(qpm[i][:, :, 18:SLOT], 0.0)

            for chi in range(n_chunks):
                pb = patch[chi % NBUF]
                qb = qpm[chi % NBUF]
                tb = qT[chi % NBUF]
                c0 = chi * ch_cols

                # Last chunks: split the gather into quarter-sized pieces so
                # their compute overlaps their own gather stream and the
                # pipeline drain after the final transfer is one quarter, not
                # a whole chunk.
                nsplit = 4 if chi >= n_chunks - 2 else 1
                gpts = CHPTS // nsplit
                for gs in range(nsplit):
                    nc.gpsimd.dma_gather(
                        out_ap=pb[:, gs * (ch_cols // nsplit):
                                  (gs + 1) * (ch_cols // nsplit), :],
                        in_ap=ptw_d[chi * WIN:(chi + 1) * WIN, :],
                        idxs_ap=idxall[:, chi * (CHPTS // 16) + gs * (gpts // 16):
                                       chi * (CHPTS // 16) + (gs + 1) * (gpts // 16)],
                        num_idxs=gpts,
                        num_idxs_reg=gpts,
                        elem_size=PTROW,
                        single_packet=False,
                    )

                # q = coef * patch (fp16); gather wrote rank r = g*128+p to
                # pb[p, g, :]; host lays c4/idf out in the same rank order.
                # Processed in chunk-halves so the transpose/MLP chain starts
                # before the whole chunk's q is built.
                qTp = pst.tile([P, n_blk, 128], f16, tag="qTp")
                qflat = qb[:].rearrange("p t s -> p (t s)")
                nh = 2 * nsplit
                hc = ch_cols // nh
                for hh in range(nh):
                    tsl = slice(hh * hc, (hh + 1) * hc)
                    cbc = c4t[:, c0 + hh * hc:c0 + (hh + 1) * hc, :].unsqueeze(
                        3).to_broadcast([P, hc, 4, 4])
                    V.tensor_tensor(
                        qb[:, tsl, 0:16].rearrange("p t (c f) -> p t c f", c=4),
                        pb[:, tsl, 0:16].rearrange("p t (c f) -> p t c f", c=4),
                        cbc,
                        AL.mult,
                    )
                    # idf -> slot 17 (w1x row 17); slot 16 is the ones row
                    V.tensor_copy(qb[:, tsl, 17:18],
                                  idft[:, c0 + hh * hc:c0 + (hh + 1) * hc
                                       ].unsqueeze(2))
                    # transpose on TensorE ([128,128] blocks into fp16 PSUM)
                    # + a DVE copy back to SBUF; keeps the DMA engines free
                    # for the gather chain.
                    for b in range(hh * n_blk // nh, (hh + 1) * n_blk // nh):
                        nc.tensor.transpose(qTp[:, b, :],
                                            qflat[:, 128 * b:128 * (b + 1)],
                                            idw[:])
                    tdst = tb[:, hh * n_blk // nh:(hh + 1) * n_blk // nh, :
                              ].rearrange("p b c -> p (b c)")
                    tsrc = qTp[:, hh * n_blk // nh:(hh + 1) * n_blk // nh, :
                               ].rearrange("p b c -> p (b c)")
                    if hh % 2 == 0:
                        V.tensor_copy(tdst, tsrc)
                    else:
                        nc.scalar.copy(tdst, tsrc)

                y16 = op_.tile([32, 2 * QCOLS], f16, tag="y16")
                l3p = ps3.tile([32, 2 * QCOLS], f32, tag="l3p")
                for qq in range(4):
                    rhs = tb[:, 4 * qq:4 * qq + 4, :]
                    # two independent 512-col half-pipelines (A on DVE relu,
                    # B on ACT relu) with 1-bank double-buffered PSUM tiles
                    # so quarters overlap across the chunk boundary.
                    l1a = ps1.tile([P, QCOLS], f32, tag="l1a")
                    l1b = ps1.tile([P, QCOLS], f32, tag="l1b")
                    nc.tensor.matmul(l1a[:], l1w[0:64], rhs[0:64],
                                     start=True, stop=True)
                    nc.tensor.matmul(l1b[:], l1w[64:128],
                                     rhs[64:128], start=True, stop=True,
                                     tile_position=(64, 0))
                    h1 = mp.tile([P, 2 * QCOLS], f16, tag="h1")
                    V.tensor_scalar(h1[:, 0:QCOLS], l1a[:], 0.0, None, AL.max)
                    nc.scalar.activation(h1[:, QCOLS:2 * QCOLS], l1b[:],
                                         mybir.ActivationFunctionType.Relu)

                    l2a = ps2.tile([P, QCOLS], f32, tag="l2a")
                    l2b = ps2.tile([P, QCOLS], f32, tag="l2b")
                    nc.tensor.matmul(l2a[:], l2w[:], h1[:, 0:QCOLS],
                                     start=True, stop=True)
                    nc.tensor.matmul(l2b[:], l2w[:], h1[:, QCOLS:2 * QCOLS],
                                     start=True, stop=True)
                    h2 = mp.tile([P, 2 * QCOLS], f16, tag="h2")
                    V.tensor_scalar(h2[:, 0:QCOLS], l2a[:],
                                    b2r[:], 0.0, AL.add, AL.max)
                    nc.scalar.activation(h2[:, QCOLS:2 * QCOLS], l2b[:],
                                         mybir.ActivationFunctionType.Relu,
                                         bias=b2r[:])

                    # quarter qq accumulates into rows 8qq..8qq+5 of l3p via a
                    # zero-padded [128, 32] lhsT variant
                    nc.tensor.matmul(l3p[:, 0:QCOLS], l3w[:, qq, :],
                                     h2[:, 0:QCOLS],
                                     start=(qq == 0), stop=(qq == 3))
                    nc.tensor.matmul(l3p[:, QCOLS:2 * QCOLS], l3w[:, qq, :],
                                     h2[:, QCOLS:2 * QCOLS],
                                     start=(qq == 0), stop=(qq == 3))

                nc.scalar.activation(y16[:], l3p[:],
                                     mybir.ActivationFunctionType.Sigmoid,
                                     bias=b3r[:])
                ccol = chi * 2 * QCOLS
                nc.sync.dma_start(out_d[:, ccol:ccol + 2 * QCOLS], y16[:])

    return nc


def _host_coefs(x):
    """Exact fp32 replica of the reference's cell/weight computation."""
    u = np.ascontiguousarray(x[:, 1], dtype=np.float32)
    v = np.ascontiguousarray(x[:, 2], dtype=np.float32)
    xu = u * np.float32(RX)
    yv = v * np.float32(RY)
    x0 = np.trunc(xu).astype(np.float32)
    y0 = np.trunc(yv).astype(np.float32)
    x0w = np.where(x0 == RX, np.float32(0.0), x0)
    wx = xu - x0w
    wy = yv - y0
    cell = (y0 * RX + x0w).astype(np.int64)
    cell = np.minimum(cell, RX * RY - 1)
    c4 = np.empty((x.shape[0], 4), np.float32)
    c4[:, 0] = (1.0 - wx) * (1.0 - wy)
    c4[:, 1] = wx * (1.0 - wy)
    c4[:, 2] = (1.0 - wx) * wy
    c4[:, 3] = wx * wy
    return cell, c4.astype(np.float16)


def _host_prep_weights(w1, b1, w2, b2, w3, b3):
    w1 = np.asarray(w1, np.float32)
    b1 = np.asarray(b1, np.float32)
    w1x = np.zeros((18, HID), np.float32)
    for c in range(4):
        w1x[4 * c:4 * c + 4, :] = w1[1:5, :]
    w1x[16, :] = b1          # slot 16 is the ones row
    w1x[17, :] = w1[0, :]    # slot 17 is idf
    lhsT1 = np.zeros((128, 128), np.float16)
    lhsT1[0:18, 0:64] = w1x
    lhsT1[32:32 + 18, 64:128] = w1x
    lhsT1[64:128, :] = lhsT1[0:64, :]
    lhsT2 = np.zeros((128, 128), np.float16)
    lhsT2[0:64, 0:64] = w2
    lhsT2[64:128, 64:128] = w2
    lhsT3 = np.zeros((128, 4, 32), np.float16)
    for qq in range(4):
        lhsT3[0:64, qq, 8 * qq:8 * qq + 3] = w3
        lhsT3[64:128, qq, 8 * qq + 3:8 * qq + 6] = w3
    b2rep = np.concatenate([b2, b2]).astype(np.float32).reshape(128, 1)
    b3rep = np.zeros((32, 1), np.float32)
    for qq in range(4):
        b3rep[8 * qq:8 * qq + 3, 0] = b3
        b3rep[8 * qq + 3:8 * qq + 6, 0] = b3
    return lhsT1.reshape(128, 128), lhsT2, lhsT3.reshape(128, 128), b2rep, b3rep


def _patch_table(emb):
    e = np.asarray(emb, dtype=np.float32).reshape(RY, RX, F)
    xs = np.arange(RX)
    x1 = np.minimum(xs + 1, RX - 1)
    ys = np.arange(RY)
    y1 = np.minimum(ys + 1, RY - 1)
    pt = np.zeros((RY, RX, PTROW), dtype=np.float16)
    pt[:, :, 0:F] = e
    pt[:, :, F:2 * F] = e[:, x1, :]
    pt[:, :, 2 * F:3 * F] = e[y1, :, :]
    pt[:, :, 3 * F:4 * F] = e[y1][:, x1, :]
    return np.ascontiguousarray(pt.reshape(RX * RY, PTROW))


def _out_maps(B):
    """(rank_local, feat, valid) lookup arrays for the [32, 1024] chunk tile.

    Row 8qq+r (r<6), col c: h=c//512, m=c%512, b=m//128, p=m%128;
    point col t = 16qq + 4b + 2h + r//3; rank = t*128 + p; feat = r%3.
    """
    rows = np.arange(32)[:, None]
    cols = np.arange(2 * QCOLS)[None, :]
    qq = rows // 8
    r = rows % 8
    h = cols // QCOLS
    m = cols % QCOLS
    b = m // 128
    p = m % 128
    t = 16 * qq + 4 * b + 2 * h + (r // 3)
    rank = t * 128 + p
    feat = np.broadcast_to(r % 3, rank.shape)
    valid = np.broadcast_to(r < 6, rank.shape)
    return rank, feat, valid


def _prep_in_maps(x, emb, w1, b1, w2, b2, w3, b3, n_cores):
    x = np.asarray(x, np.float32)
    N = x.shape[0]
    B = N // n_cores
    T = B // P
    n_chunks = B // CHPTS
    cell, c4 = _host_coefs(x)
    order = np.argsort(cell, kind="stable")
    cell_s = cell[order]
    xs_idf = x[order, 0].astype(np.float16)
    c4_s = c4[order]
    pt = _patch_table(emb)
    lhsT1, lhsT2, lhsT3, b2rep, b3rep = _host_prep_weights(w1, b1, w2, b2, w3, b3)
    in_maps = []
    for k in range(n_cores):
        ci = cell_s[k * B:(k + 1) * B]
        ptw = np.empty((n_chunks * WIN, PTROW), np.float16)
        idx16 = np.empty((P, n_chunks * CHPTS // 16), np.int16)
        for c in range(n_chunks):
            cc = ci[c * CHPTS:(c + 1) * CHPTS]
            base = int(np.clip((int(cc[0]) + int(cc[-1]) + 1) // 2 - WIN // 2,
                               0, RX * RY - WIN))
            lo = cc - base
            assert lo.min() >= 0 and lo.max() < WIN, (
                f"window miss core {k} chunk {c}: {lo.min()} {lo.max()}")
            ptw[c * WIN:(c + 1) * WIN] = pt[base:base + WIN]
            w16 = lo.astype(np.int16).reshape(CHPTS // 16, 16).T
            idx16[:, c * (CHPTS // 16):(c + 1) * (CHPTS // 16)] = np.tile(
                w16, (8, 1))
        # rank r = t*128 + p -> [p, t] column-major partition layout
        c4pm = np.ascontiguousarray(
            c4_s[k * B:(k + 1) * B].reshape(T, P, 4).transpose(1, 0, 2)
        ).reshape(P, T * 4)
        idfpm = np.ascontiguousarray(
            xs_idf[k * B:(k + 1) * B].reshape(T, P).T)
        in_maps.append({
            "c4pm": c4pm,
            "idfpm": idfpm,
            "ptw": ptw,
            "idx16": idx16,
            "lhsT1": lhsT1,
            "lhsT2": lhsT2,
            "lhsT3": lhsT3,
            "ident": np.eye(128, dtype=np.float16),
            "b2rep": b2rep,
            "b3rep": b3rep,
        })
    return in_maps, order


_CACHE = {}


def kernel(x, emb, w1, b1, w2, b2, w3, b3):
    from concourse.bass_utils import run_bass_kernel_spmd

    x = np.asarray(x, np.float32)
    N = x.shape[0]
    B = N // N_CORES
    n_chunks = B // CHPTS

    in_maps, order = _prep_in_maps(x, emb, w1, b1, w2, b2, w3, b3,
                                   n_cores=N_CORES)

    key = (B,)
    if key not in _CACHE:
        nc_new = _build_bass(B)
        nc_new.compile()
        _CACHE[key] = nc_new
    nc = _CACHE[key]

    trace = os.environ.get("KERNEL_TRACE", "0") == "1"
    res = run_bass_kernel_spmd(
        nc, in_maps, core_ids=list(range(N_CORES)), trace=trace
    )
    if trace and res.exec_time_ns is not None:
        print(f"HW exec time: {res.exec_time_ns} ns")

    rank_l, feat, valid = _out_maps(B)
    rows, cols = np.nonzero(valid)
    rk = rank_l[rows, cols]
    ft = feat[rows, cols]
    y_sorted = np.empty((N, 3), np.float32)
    for k in range(N_CORES):
        yT = np.asarray(res.results[k]["yT"], np.float32)
        yc = yT.reshape(32, n_chunks, 2 * QCOLS).transpose(1, 0, 2)
        base = k * B + np.arange(n_chunks)[:, None] * CHPTS
        y_sorted[base + rk[None, :], np.broadcast_to(ft, (n_chunks, ft.size))] = (
            yc[:, rows, cols])
    y = np.empty((N, 3), np.float32)
    y[order, :] = y_sorted
    return y


# revision 23
# speedup vs baseline: 1.0340x; 1.0340x over previous
# kernel.py — Trainium2 Bass kernel for nn_DenseGridNet (bilinear grid sample + MLP)
#
# Strategy (data-parallel over 8 NeuronCores, sorted sharding):
#  * host: computes cell ids + bilinear coefs (exact fp32 replica of the
#    reference), sorts points by cell, builds an fp16 patch table with
#    256B rows PT[cell] = [v00,v10,v01,v11, pad...] (dma_gather needs
#    256B-aligned elements) and per-chunk 8192-row windows with int16
#    window-local indices.
#  * device, per 8192-point chunk: bulk dma_gather of 256B fp16 patch
#    rows; DVE multiplies the 4 bilinear coefs (host-sent, fp16) into the
#    16 patch values; one xbar DMA transpose flips point-major ->
#    feature-major; TensorE runs the 3 MLP layers as block-diagonal fp16
#    matmuls (2 pts/col); relu is split DVE/ACT; the third layer
#    accumulates all 4 quarters into one [32, 1024] PSUM tile via
#    zero-padded lhsT variants so a single sigmoid serves the chunk;
#    output lands as fp16 yT [32, n_chunks*1024] and is un-permuted on
#    host.
import os
import numpy as np

RX = 1024
RY = 1024
F = 4
HID = 64
N_CORES = 8
P = 128          # partitions
SLOT = 32        # fp16 columns per point in the pre-transpose buffer
WIN = 8192       # dma_gather window rows (int16-addressable)
CHPTS = 8192     # points per gather/transpose chunk
PTROW = 128      # padded patch row, fp16 elements (256B)
QCOLS = 512
XS = 544         # DVE/ACT relu column split (of 1024)


def _build_bass(B):
    """Bass program for one core processing B sorted points."""
    import concourse.bass as bass
    import concourse.tile as tile
    from concourse import bacc, library_config
    import concourse.mybir as mybir

    dt = mybir.dt
    T = B // P
    n_chunks = B // CHPTS
    ch_cols = CHPTS // P            # 64 point cols per chunk
    n_blk = ch_cols * SLOT // 128   # 16 transposed blocks per chunk

    nc = bacc.Bacc(None, target_bir_lowering=False)

    f32 = dt.float32
    f16 = dt.float16

    # ---- DRAM I/O -------------------------------------------------------
    c4_d = nc.dram_tensor("c4pm", [P, T * 4], f16, kind="ExternalInput")
    idf_d = nc.dram_tensor("idfpm", [P, T], f16, kind="ExternalInput")
    ptw_d = nc.dram_tensor("ptw", [n_chunks * WIN, PTROW], f16,
                           kind="ExternalInput")
    idx_d = nc.dram_tensor("idx16", [P, n_chunks * CHPTS // 16], dt.int16,
                           kind="ExternalInput")
    l1_d = nc.dram_tensor("lhsT1", [128, 128], f16, kind="ExternalInput")
    l2_d = nc.dram_tensor("lhsT2", [128, 128], f16, kind="ExternalInput")
    l3_d = nc.dram_tensor("lhsT3", [128, 128], f16, kind="ExternalInput")
    id_d = nc.dram_tensor("ident", [128, 128], f16, kind="ExternalInput")
    b2_d = nc.dram_tensor("b2rep", [128, 1], f32, kind="ExternalInput")
    b3_d = nc.dram_tensor("b3rep", [32, 1], f32, kind="ExternalInput")
    out_d = nc.dram_tensor("yT", [32, n_chunks * 2 * QCOLS], f16,
                           kind="ExternalOutput")

    with tile.TileContext(nc) as tc:
        with (
            tc.tile_pool(name="persist", bufs=1) as pp,
            tc.tile_pool(name="psum_l1", bufs=1, space="PSUM") as ps1,
            tc.tile_pool(name="psum_l2", bufs=1, space="PSUM") as ps2,
            tc.tile_pool(name="psum_l3", bufs=1, space="PSUM") as ps3,
            tc.tile_pool(name="psum_tr", bufs=1, space="PSUM") as pst,
            tc.tile_pool(name="mlp", bufs=4) as mp,
            tc.tile_pool(name="outp", bufs=3) as op_,
        ):
            AL = mybir.AluOpType
            V = nc.vector
            nc.gpsimd.load_library(library_config.mlp)

            c4t = pp.tile([P, T, 4], f16, tag="c4t")
            idft = pp.tile([P, T], f16, tag="idft")
            l1w = pp.tile([128, 128], f16, tag="l1w")
            l2w = pp.tile([128, 128], f16, tag="l2w")
            l3w = pp.tile([128, 4, 32], f16, tag="l3w")
            idw = pp.tile([128, 128], f16, tag="idw")
            b2r = pp.tile([128, 1], f32, tag="b2r")
            b3r = pp.tile([32, 1], f32, tag="b3r")

            NBUF = 2
            idxall = pp.tile([P, n_chunks * CHPTS // 16], dt.int16,
                             tag="idxall")
            patch = [pp.tile([P, ch_cols, PTROW], f16, name=f"patch{i}",
                             tag=f"patch{i}") for i in range(NBUF)]
            qpm = [pp.tile([P, ch_cols, SLOT], f16, name=f"qpm{i}",
                           tag=f"qpm{i}") for i in range(NBUF)]
            qT = [pp.tile([P, n_blk, 128], f16, name=f"qT{i}", tag=f"qT{i}")
                  for i in range(NBUF)]

            # idx first (small piece, then the rest) so chunk-0 desc-gen
            # starts early; remaining loads queue behind the first gathers.
            IW = CHPTS // 16
            nc.sync.dma_start(idxall[:, 0:2 * IW], idx_d[:, 0:2 * IW])
            nc.sync.dma_start(idxall[:, 2 * IW:8 * IW], idx_d[:, 2 * IW:8 * IW])
            nc.sync.dma_start(idxall[:, 8 * IW:], idx_d[:, 8 * IW:])
            nc.sync.dma_start(c4t[:].rearrange("p t c -> p (t c)"), c4_d[:])
            nc.sync.dma_start(idft[:], idf_d[:])
            nc.sync.dma_start(l1w[:], l1_d[:])
            nc.sync.dma_start(l2w[:], l2_d[:])
            nc.sync.dma_start(l3w[:].rearrange("p a b -> p (a b)"), l3_d[:])
            nc.sync.dma_start(idw[:], id_d[:])
            nc.sync.dma_start(b2r[:], b2_d[:])
            nc.sync.dma_start(b3r[:], b3_d[:])

            for i in range(NBUF):
                nc.vector.memset(qpm[i][:, :, 16:17], 1.0)
                nc.vector.memset(qpm[i][:, :, 18:SLOT], 0.0)

            for chi in range(n_chunks):
                pb = patch[chi % NBUF]
                qb = qpm[chi % NBUF]
                tb = qT[chi % NBUF]
                c0 = chi * ch_cols

                # Last chunks: split the gather into quarter-sized pieces so
                # their compute overlaps their own gather stream and the
                # pipeline drain after the final transfer is one quarter, not
                # a whole chunk.
                nsplit = 4 if chi >= n_chunks - 2 else 1
                gpts = CHPTS // nsplit
                for gs in range(nsplit):
                    nc.gpsimd.dma_gather(
                        out_ap=pb[:, gs * (ch_cols // nsplit):
                                  (gs + 1) * (ch_cols // nsplit), :],
                        in_ap=ptw_d[chi * WIN:(chi + 1) * WIN, :],
                        idxs_ap=idxall[:, chi * (CHPTS // 16) + gs * (gpts // 16):
                                       chi * (CHPTS // 16) + (gs + 1) * (gpts // 16)],
                        num_idxs=gpts,
                        num_idxs_reg=gpts,
                        elem_size=PTROW,
                        single_packet=False,
                    )

                # q = coef * patch (fp16); gather wrote rank r = g*128+p to
                # pb[p, g, :]; host lays c4/idf out in the same rank order.
                # Processed in chunk-halves so the transpose/MLP chain starts
                # before the whole chunk's q is built.
                qTp = pst.tile([P, n_blk, 128], f16, tag="qTp")
                qflat = qb[:].rearrange("p t s -> p (t s)")
                nh = 2 * nsplit
                hc = ch_cols // nh
                for hh in range(nh):
                    tsl = slice(hh * hc, (hh + 1) * hc)
                    cbc = c4t[:, c0 + hh * hc:c0 + (hh + 1) * hc, :].unsqueeze(
                        3).to_broadcast([P, hc, 4, 4])
                    V.tensor_tensor(
                        qb[:, tsl, 0:16].rearrange("p t (c f) -> p t c f", c=4),
                        pb[:, tsl, 0:16].rearrange("p t (c f) -> p t c f", c=4),
                        cbc,
                        AL.mult,
                    )
                    # idf -> slot 17 (w1x row 17); slot 16 is the ones row
                    V.tensor_copy(qb[:, tsl, 17:18],
                                  idft[:, c0 + hh * hc:c0 + (hh + 1) * hc
                                       ].unsqueeze(2))
                    # transpose on TensorE ([128,128] blocks into fp16 PSUM)
                    # + a DVE copy back to SBUF; keeps the DMA engines free
                    # for the gather chain.
                    for b in range(hh * n_blk // nh, (hh + 1) * n_blk // nh):
                        nc.tensor.transpose(qTp[:, b, :],
                                            qflat[:, 128 * b:128 * (b + 1)],
                                            idw[:])
                    V.tensor_copy(
                        tb[:, hh * n_blk // nh:(hh + 1) * n_blk // nh, :
                           ].rearrange("p b c -> p (b c)"),
                        qTp[:, hh * n_blk // nh:(hh + 1) * n_blk // nh, :
                            ].rearrange("p b c -> p (b c)"))

                y16 = op_.tile([32, 2 * QCOLS], f16, tag="y16")
                l3p = ps3.tile([32, 2 * QCOLS], f32, tag="l3p")
                for qq in range(4):
                    rhs = tb[:, 4 * qq:4 * qq + 4, :]
                    # two independent 512-col half-pipelines (A on DVE relu,
                    # B on ACT relu) with 1-bank double-buffered PSUM tiles
                    # so quarters overlap across the chunk boundary.
                    l1a = ps1.tile([P, QCOLS], f32, tag="l1a")
                    l1b = ps1.tile([P, QCOLS], f32, tag="l1b")
                    nc.tensor.matmul(l1a[:], l1w[0:64], rhs[0:64],
                                     start=True, stop=True)
                    nc.tensor.matmul(l1b[:], l1w[64:128],
                                     rhs[64:128], start=True, stop=True,
                                     tile_position=(64, 0))
                    h1 = mp.tile([P, 2 * QCOLS], f16, tag="h1")
                    V.tensor_scalar(h1[:, 0:QCOLS], l1a[:], 0.0, None, AL.max)
                    nc.scalar.activation(h1[:, QCOLS:2 * QCOLS], l1b[:],
                                         mybir.ActivationFunctionType.Relu)

                    l2a = ps2.tile([P, QCOLS], f32, tag="l2a")
                    l2b = ps2.tile([P, QCOLS], f32, tag="l2b")
                    nc.tensor.matmul(l2a[:], l2w[:], h1[:, 0:QCOLS],
                                     start=True, stop=True)
                    nc.tensor.matmul(l2b[:], l2w[:], h1[:, QCOLS:2 * QCOLS],
                                     start=True, stop=True)
                    h2 = mp.tile([P, 2 * QCOLS], f16, tag="h2")
                    V.tensor_scalar(h2[:, 0:QCOLS], l2a[:],
                                    b2r[:], 0.0, AL.add, AL.max)
                    nc.scalar.activation(h2[:, QCOLS:2 * QCOLS], l2b[:],
                                         mybir.ActivationFunctionType.Relu,
                                         bias=b2r[:])

                    # quarter qq accumulates into rows 8qq..8qq+5 of l3p via a
                    # zero-padded [128, 32] lhsT variant
                    nc.tensor.matmul(l3p[:, 0:QCOLS], l3w[:, qq, :],
                                     h2[:, 0:QCOLS],
                                     start=(qq == 0), stop=(qq == 3))
                    nc.tensor.matmul(l3p[:, QCOLS:2 * QCOLS], l3w[:, qq, :],
                                     h2[:, QCOLS:2 * QCOLS],
                                     start=(qq == 0), stop=(qq == 3))

                nc.scalar.activation(y16[:], l3p[:],
                                     mybir.ActivationFunctionType.Sigmoid,
                                     bias=b3r[:])
                ccol = chi * 2 * QCOLS
                nc.sync.dma_start(out_d[:, ccol:ccol + 2 * QCOLS], y16[:])

    return nc


def _host_coefs(x):
    """Exact fp32 replica of the reference's cell/weight computation."""
    u = np.ascontiguousarray(x[:, 1], dtype=np.float32)
    v = np.ascontiguousarray(x[:, 2], dtype=np.float32)
    xu = u * np.float32(RX)
    yv = v * np.float32(RY)
    x0 = np.trunc(xu).astype(np.float32)
    y0 = np.trunc(yv).astype(np.float32)
    x0w = np.where(x0 == RX, np.float32(0.0), x0)
    wx = xu - x0w
    wy = yv - y0
    cell = (y0 * RX + x0w).astype(np.int64)
    cell = np.minimum(cell, RX * RY - 1)
    c4 = np.empty((x.shape[0], 4), np.float32)
    c4[:, 0] = (1.0 - wx) * (1.0 - wy)
    c4[:, 1] = wx * (1.0 - wy)
    c4[:, 2] = (1.0 - wx) * wy
    c4[:, 3] = wx * wy
    return cell, c4.astype(np.float16)


def _host_prep_weights(w1, b1, w2, b2, w3, b3):
    w1 = np.asarray(w1, np.float32)
    b1 = np.asarray(b1, np.float32)
    w1x = np.zeros((18, HID), np.float32)
    for c in range(4):
        w1x[4 * c:4 * c + 4, :] = w1[1:5, :]
    w1x[16, :] = b1          # slot 16 is the ones row
    w1x[17, :] = w1[0, :]    # slot 17 is idf
    lhsT1 = np.zeros((128, 128), np.float16)
    lhsT1[0:18, 0:64] = w1x
    lhsT1[32:32 + 18, 64:128] = w1x
    lhsT1[64:128, :] = lhsT1[0:64, :]
    lhsT2 = np.zeros((128, 128), np.float16)
    lhsT2[0:64, 0:64] = w2
    lhsT2[64:128, 64:128] = w2
    lhsT3 = np.zeros((128, 4, 32), np.float16)
    for qq in range(4):
        lhsT3[0:64, qq, 8 * qq:8 * qq + 3] = w3
        lhsT3[64:128, qq, 8 * qq + 3:8 * qq + 6] = w3
    b2rep = np.concatenate([b2, b2]).astype(np.float32).reshape(128, 1)
    b3rep = np.zeros((32, 1), np.float32)
    for qq in range(4):
        b3rep[8 * qq:8 * qq + 3, 0] = b3
        b3rep[8 * qq + 3:8 * qq + 6, 0] = b3
    return lhsT1.reshape(128, 128), lhsT2, lhsT3.reshape(128, 128), b2rep, b3rep


def _patch_table(emb):
    e = np.asarray(emb, dtype=np.float32).reshape(RY, RX, F)
    xs = np.arange(RX)
    x1 = np.minimum(xs + 1, RX - 1)
    ys = np.arange(RY)
    y1 = np.minimum(ys + 1, RY - 1)
    pt = np.zeros((RY, RX, PTROW), dtype=np.float16)
    pt[:, :, 0:F] = e
    pt[:, :, F:2 * F] = e[:, x1, :]
    pt[:, :, 2 * F:3 * F] = e[y1, :, :]
    pt[:, :, 3 * F:4 * F] = e[y1][:, x1, :]
    return np.ascontiguousarray(pt.reshape(RX * RY, PTROW))


def _out_maps(B):
    """(rank_local, feat, valid) lookup arrays for the [32, 1024] chunk tile.

    Row 8qq+r (r<6), col c: h=c//512, m=c%512, b=m//128, p=m%128;
    point col t = 16qq + 4b + 2h + r//3; rank = t*128 + p; feat = r%3.
    """
    rows = np.arange(32)[:, None]
    cols = np.arange(2 * QCOLS)[None, :]
    qq = rows // 8
    r = rows % 8
    h = cols // QCOLS
    m = cols % QCOLS
    b = m // 128
    p = m % 128
    t = 16 * qq + 4 * b + 2 * h + (r // 3)
    rank = t * 128 + p
    feat = np.broadcast_to(r % 3, rank.shape)
    valid = np.broadcast_to(r < 6, rank.shape)
    return rank, feat, valid


def _prep_in_maps(x, emb, w1, b1, w2, b2, w3, b3, n_cores):
    x = np.asarray(x, np.float32)
    N = x.shape[0]
    B = N // n_cores
    T = B // P
    n_chunks = B // CHPTS
    cell, c4 = _host_coefs(x)
    order = np.argsort(cell, kind="stable")
    cell_s = cell[order]
    xs_idf = x[order, 0].astype(np.float16)
    c4_s = c4[order]
    pt = _patch_table(emb)
    lhsT1, lhsT2, lhsT3, b2rep, b3rep = _host_prep_weights(w1, b1, w2, b2, w3, b3)
    in_maps = []
    for k in range(n_cores):
        ci = cell_s[k * B:(k + 1) * B]
        ptw = np.empty((n_chunks * WIN, PTROW), np.float16)
        idx16 = np.empty((P, n_chunks * CHPTS // 16), np.int16)
        for c in range(n_chunks):
            cc = ci[c * CHPTS:(c + 1) * CHPTS]
            base = int(np.clip((int(cc[0]) + int(cc[-1]) + 1) // 2 - WIN // 2,
                               0, RX * RY - WIN))
            lo = cc - base
            assert lo.min() >= 0 and lo.max() < WIN, (
                f"window miss core {k} chunk {c}: {lo.min()} {lo.max()}")
            ptw[c * WIN:(c + 1) * WIN] = pt[base:base + WIN]
            w16 = lo.astype(np.int16).reshape(CHPTS // 16, 16).T
            idx16[:, c * (CHPTS // 16):(c + 1) * (CHPTS // 16)] = np.tile(
                w16, (8, 1))
        # rank r = t*128 + p -> [p, t] column-major partition layout
        c4pm = np.ascontiguousarray(
            c4_s[k * B:(k + 1) * B].reshape(T, P, 4).transpose(1, 0, 2)
        ).reshape(P, T * 4)
        idfpm = np.ascontiguousarray(
            xs_idf[k * B:(k + 1) * B].reshape(T, P).T)
        in_maps.append({
            "c4pm": c4pm,
            "idfpm": idfpm,
            "ptw": ptw,
            "idx16": idx16,
            "lhsT1": lhsT1,
            "lhsT2": lhsT2,
            "lhsT3": lhsT3,
            "ident": np.eye(128, dtype=np.float16),
            "b2rep": b2rep,
            "b3rep": b3rep,
        })
    return in_maps, order


_CACHE = {}


def kernel(x, emb, w1, b1, w2, b2, w3, b3):
    from concourse.bass_utils import run_bass_kernel_spmd

    x = np.asarray(x, np.float32)
    N = x.shape[0]
    B = N // N_CORES
    n_chunks = B // CHPTS

    in_maps, order = _prep_in_maps(x, emb, w1, b1, w2, b2, w3, b3,
                                   n_cores=N_CORES)

    key = (B,)
    if key not in _CACHE:
        nc_new = _build_bass(B)
        nc_new.compile()
        _CACHE[key] = nc_new
    nc = _CACHE[key]

    trace = os.environ.get("KERNEL_TRACE", "0") == "1"
    res = run_bass_kernel_spmd(
        nc, in_maps, core_ids=list(range(N_CORES)), trace=trace
    )
    if trace and res.exec_time_ns is not None:
        print(f"HW exec time: {res.exec_time_ns} ns")

    rank_l, feat, valid = _out_maps(B)
    rows, cols = np.nonzero(valid)
    rk = rank_l[rows, cols]
    ft = feat[rows, cols]
    y_sorted = np.empty((N, 3), np.float32)
    for k in range(N_CORES):
        yT = np.asarray(res.results[k]["yT"], np.float32)
        yc = yT.reshape(32, n_chunks, 2 * QCOLS).transpose(1, 0, 2)
        base = k * B + np.arange(n_chunks)[:, None] * CHPTS
        y_sorted[base + rk[None, :], np.broadcast_to(ft, (n_chunks, ft.size))] = (
            yc[:, rows, cols])
    y = np.empty((N, 3), np.float32)
    y[order, :] = y_sorted
    return y


# revision 24
# speedup vs baseline: 1.0492x; 1.0147x over previous
# kernel.py — Trainium2 Bass kernel for nn_DenseGridNet (bilinear grid sample + MLP)
#
# Strategy (data-parallel over 8 NeuronCores, sorted sharding):
#  * host: computes cell ids + bilinear coefs (exact fp32 replica of the
#    reference), sorts points by cell, builds an fp16 patch table with
#    256B rows PT[cell] = [v00,v10,v01,v11, pad...] (dma_gather needs
#    256B-aligned elements) and per-chunk 8192-row windows with int16
#    window-local indices.
#  * device, per 8192-point chunk: bulk dma_gather of 256B fp16 patch
#    rows; DVE multiplies the 4 bilinear coefs (host-sent, fp16) into the
#    16 patch values; one xbar DMA transpose flips point-major ->
#    feature-major; TensorE runs the 3 MLP layers as block-diagonal fp16
#    matmuls (2 pts/col); relu is split DVE/ACT; the third layer
#    accumulates all 4 quarters into one [32, 1024] PSUM tile via
#    zero-padded lhsT variants so a single sigmoid serves the chunk;
#    output lands as fp16 yT [32, n_chunks*1024] and is un-permuted on
#    host.
import os
import numpy as np

RX = 1024
RY = 1024
F = 4
HID = 64
N_CORES = 8
P = 128          # partitions
SLOT = 32        # fp16 columns per point in the pre-transpose buffer
WIN = 8192       # dma_gather window rows (int16-addressable)
CHPTS = 8192     # points per gather/transpose chunk
PTROW = 128      # padded patch row, fp16 elements (256B)
QCOLS = 512
XS = 544         # DVE/ACT relu column split (of 1024)


def _build_bass(B):
    """Bass program for one core processing B sorted points."""
    import concourse.bass as bass
    import concourse.tile as tile
    from concourse import bacc, library_config
    import concourse.mybir as mybir

    dt = mybir.dt
    T = B // P
    n_chunks = B // CHPTS
    ch_cols = CHPTS // P            # 64 point cols per chunk
    n_blk = ch_cols * SLOT // 128   # 16 transposed blocks per chunk

    nc = bacc.Bacc(None, target_bir_lowering=False)

    f32 = dt.float32
    f16 = dt.float16

    # ---- DRAM I/O -------------------------------------------------------
    c4_d = nc.dram_tensor("c4pm", [P, T * 4], f16, kind="ExternalInput")
    idf_d = nc.dram_tensor("idfpm", [P, T], f16, kind="ExternalInput")
    ptw_d = nc.dram_tensor("ptw", [n_chunks * WIN, PTROW], f16,
                           kind="ExternalInput")
    idx_d = nc.dram_tensor("idx16", [16, n_chunks * CHPTS // 16], dt.int16,
                           kind="ExternalInput")
    l1_d = nc.dram_tensor("lhsT1", [128, 128], f16, kind="ExternalInput")
    l2_d = nc.dram_tensor("lhsT2", [128, 128], f16, kind="ExternalInput")
    l3_d = nc.dram_tensor("lhsT3", [128, 128], f16, kind="ExternalInput")
    id_d = nc.dram_tensor("ident", [128, 128], f16, kind="ExternalInput")
    b2_d = nc.dram_tensor("b2rep", [128, 1], f32, kind="ExternalInput")
    b3_d = nc.dram_tensor("b3rep", [32, 1], f32, kind="ExternalInput")
    out_d = nc.dram_tensor("yT", [32, n_chunks * 2 * QCOLS], f16,
                           kind="ExternalOutput")

    with tile.TileContext(nc) as tc:
        with (
            tc.tile_pool(name="persist", bufs=1) as pp,
            tc.tile_pool(name="psum_l1", bufs=1, space="PSUM") as ps1,
            tc.tile_pool(name="psum_l2", bufs=1, space="PSUM") as ps2,
            tc.tile_pool(name="psum_l3", bufs=1, space="PSUM") as ps3,
            tc.tile_pool(name="psum_tr", bufs=1, space="PSUM") as pst,
            tc.tile_pool(name="mlp", bufs=4) as mp,
            tc.tile_pool(name="outp", bufs=3) as op_,
        ):
            AL = mybir.AluOpType
            V = nc.vector
            nc.gpsimd.load_library(library_config.mlp)

            c4t = pp.tile([P, T, 4], f16, tag="c4t")
            idft = pp.tile([P, T], f16, tag="idft")
            l1w = pp.tile([128, 128], f16, tag="l1w")
            l2w = pp.tile([128, 128], f16, tag="l2w")
            l3w = pp.tile([128, 4, 32], f16, tag="l3w")
            idw = pp.tile([128, 128], f16, tag="idw")
            b2r = pp.tile([128, 1], f32, tag="b2r")
            b3r = pp.tile([32, 1], f32, tag="b3r")

            NBUF = 2
            idxall = pp.tile([P, n_chunks * CHPTS // 16], dt.int16,
                             tag="idxall")
            patch = [pp.tile([P, ch_cols, PTROW], f16, name=f"patch{i}",
                             tag=f"patch{i}") for i in range(NBUF)]
            qpm = [pp.tile([P, ch_cols, SLOT], f16, name=f"qpm{i}",
                           tag=f"qpm{i}") for i in range(NBUF)]
            qT = [pp.tile([P, n_blk, 128], f16, name=f"qT{i}", tag=f"qT{i}")
                  for i in range(NBUF)]

            # idx rows 0:16 only (the gather ucode reads the 16-partition
            # wrap); rows 16:128 are zeroed once on ACT so every partition
            # holds valid window indices.
            nc.scalar.memzero(idxall[16:128, :])
            IW = CHPTS // 16
            nc.sync.dma_start(idxall[0:16, 0:2 * IW], idx_d[:, 0:2 * IW])
            nc.sync.dma_start(idxall[0:16, 2 * IW:], idx_d[:, 2 * IW:])
            nc.sync.dma_start(c4t[:].rearrange("p t c -> p (t c)"), c4_d[:])
            nc.sync.dma_start(idft[:], idf_d[:])
            nc.sync.dma_start(l1w[:], l1_d[:])
            nc.sync.dma_start(l2w[:], l2_d[:])
            nc.sync.dma_start(l3w[:].rearrange("p a b -> p (a b)"), l3_d[:])
            nc.sync.dma_start(idw[:], id_d[:])
            nc.sync.dma_start(b2r[:], b2_d[:])
            nc.sync.dma_start(b3r[:], b3_d[:])

            for i in range(NBUF):
                nc.vector.memset(qpm[i][:, :, 16:17], 1.0)
                nc.vector.memset(qpm[i][:, :, 18:SLOT], 0.0)

            for chi in range(n_chunks):
                pb = patch[chi % NBUF]
                qb = qpm[chi % NBUF]
                tb = qT[chi % NBUF]
                c0 = chi * ch_cols

                # Last chunks: split the gather into quarter-sized pieces so
                # their compute overlaps their own gather stream and the
                # pipeline drain after the final transfer is one quarter, not
                # a whole chunk.
                nsplit = 4 if chi >= n_chunks - 2 else 1
                gpts = CHPTS // nsplit
                for gs in range(nsplit):
                    nc.gpsimd.dma_gather(
                        out_ap=pb[:, gs * (ch_cols // nsplit):
                                  (gs + 1) * (ch_cols // nsplit), :],
                        in_ap=ptw_d[chi * WIN:(chi + 1) * WIN, :],
                        idxs_ap=idxall[:, chi * (CHPTS // 16) + gs * (gpts // 16):
                                       chi * (CHPTS // 16) + (gs + 1) * (gpts // 16)],
                        num_idxs=gpts,
                        num_idxs_reg=gpts,
                        elem_size=PTROW,
                        single_packet=False,
                    )

                # q = coef * patch (fp16); gather wrote rank r = g*128+p to
                # pb[p, g, :]; host lays c4/idf out in the same rank order.
                # Processed in chunk-halves so the transpose/MLP chain starts
                # before the whole chunk's q is built.
                qTp = pst.tile([P, n_blk, 128], f16, tag="qTp")
                qflat = qb[:].rearrange("p t s -> p (t s)")
                nh = 2 * nsplit
                hc = ch_cols // nh
                for hh in range(nh):
                    tsl = slice(hh * hc, (hh + 1) * hc)
                    cbc = c4t[:, c0 + hh * hc:c0 + (hh + 1) * hc, :].unsqueeze(
                        3).to_broadcast([P, hc, 4, 4])
                    V.tensor_tensor(
                        qb[:, tsl, 0:16].rearrange("p t (c f) -> p t c f", c=4),
                        pb[:, tsl, 0:16].rearrange("p t (c f) -> p t c f", c=4),
                        cbc,
                        AL.mult,
                    )
                    # idf -> slot 17 (w1x row 17); slot 16 is the ones row
                    V.tensor_copy(qb[:, tsl, 17:18],
                                  idft[:, c0 + hh * hc:c0 + (hh + 1) * hc
                                       ].unsqueeze(2))
                    # transpose on TensorE ([128,128] blocks into fp16 PSUM)
                    # + a DVE copy back to SBUF; keeps the DMA engines free
                    # for the gather chain.
                    for b in range(hh * n_blk // nh, (hh + 1) * n_blk // nh):
                        nc.tensor.transpose(qTp[:, b, :],
                                            qflat[:, 128 * b:128 * (b + 1)],
                                            idw[:])
                    V.tensor_copy(
                        tb[:, hh * n_blk // nh:(hh + 1) * n_blk // nh, :
                           ].rearrange("p b c -> p (b c)"),
                        qTp[:, hh * n_blk // nh:(hh + 1) * n_blk // nh, :
                            ].rearrange("p b c -> p (b c)"))

                y16 = op_.tile([32, 2 * QCOLS], f16, tag="y16")
                l3p = ps3.tile([32, 2 * QCOLS], f32, tag="l3p")
                for qq in range(4):
                    rhs = tb[:, 4 * qq:4 * qq + 4, :]
                    # two independent 512-col half-pipelines (A on DVE relu,
                    # B on ACT relu) with 1-bank double-buffered PSUM tiles
                    # so quarters overlap across the chunk boundary.
                    l1a = ps1.tile([P, QCOLS], f32, tag="l1a")
                    l1b = ps1.tile([P, QCOLS], f32, tag="l1b")
                    nc.tensor.matmul(l1a[:], l1w[0:64], rhs[0:64],
                                     start=True, stop=True)
                    nc.tensor.matmul(l1b[:], l1w[64:128],
                                     rhs[64:128], start=True, stop=True,
                                     tile_position=(64, 0))
                    h1 = mp.tile([P, 2 * QCOLS], f16, tag="h1")
                    V.tensor_scalar(h1[:, 0:QCOLS], l1a[:], 0.0, None, AL.max)
                    nc.scalar.activation(h1[:, QCOLS:2 * QCOLS], l1b[:],
                                         mybir.ActivationFunctionType.Relu)

                    l2a = ps2.tile([P, QCOLS], f32, tag="l2a")
                    l2b = ps2.tile([P, QCOLS], f32, tag="l2b")
                    nc.tensor.matmul(l2a[:], l2w[:], h1[:, 0:QCOLS],
                                     start=True, stop=True)
                    nc.tensor.matmul(l2b[:], l2w[:], h1[:, QCOLS:2 * QCOLS],
                                     start=True, stop=True)
                    h2 = mp.tile([P, 2 * QCOLS], f16, tag="h2")
                    V.tensor_scalar(h2[:, 0:QCOLS], l2a[:],
                                    b2r[:], 0.0, AL.add, AL.max)
                    nc.scalar.activation(h2[:, QCOLS:2 * QCOLS], l2b[:],
                                         mybir.ActivationFunctionType.Relu,
                                         bias=b2r[:])

                    # quarter qq accumulates into rows 8qq..8qq+5 of l3p via a
                    # zero-padded [128, 32] lhsT variant
                    nc.tensor.matmul(l3p[:, 0:QCOLS], l3w[:, qq, :],
                                     h2[:, 0:QCOLS],
                                     start=(qq == 0), stop=(qq == 3))
                    nc.tensor.matmul(l3p[:, QCOLS:2 * QCOLS], l3w[:, qq, :],
                                     h2[:, QCOLS:2 * QCOLS],
                                     start=(qq == 0), stop=(qq == 3))

                nc.scalar.activation(y16[:], l3p[:],
                                     mybir.ActivationFunctionType.Sigmoid,
                                     bias=b3r[:])
                ccol = chi * 2 * QCOLS
                nc.sync.dma_start(out_d[:, ccol:ccol + 2 * QCOLS], y16[:])

    return nc


def _host_coefs(x):
    """Exact fp32 replica of the reference's cell/weight computation."""
    u = np.ascontiguousarray(x[:, 1], dtype=np.float32)
    v = np.ascontiguousarray(x[:, 2], dtype=np.float32)
    xu = u * np.float32(RX)
    yv = v * np.float32(RY)
    x0 = np.trunc(xu).astype(np.float32)
    y0 = np.trunc(yv).astype(np.float32)
    x0w = np.where(x0 == RX, np.float32(0.0), x0)
    wx = xu - x0w
    wy = yv - y0
    cell = (y0 * RX + x0w).astype(np.int64)
    cell = np.minimum(cell, RX * RY - 1)
    c4 = np.empty((x.shape[0], 4), np.float32)
    c4[:, 0] = (1.0 - wx) * (1.0 - wy)
    c4[:, 1] = wx * (1.0 - wy)
    c4[:, 2] = (1.0 - wx) * wy
    c4[:, 3] = wx * wy
    return cell, c4.astype(np.float16)


def _host_prep_weights(w1, b1, w2, b2, w3, b3):
    w1 = np.asarray(w1, np.float32)
    b1 = np.asarray(b1, np.float32)
    w1x = np.zeros((18, HID), np.float32)
    for c in range(4):
        w1x[4 * c:4 * c + 4, :] = w1[1:5, :]
    w1x[16, :] = b1          # slot 16 is the ones row
    w1x[17, :] = w1[0, :]    # slot 17 is idf
    lhsT1 = np.zeros((128, 128), np.float16)
    lhsT1[0:18, 0:64] = w1x
    lhsT1[32:32 + 18, 64:128] = w1x
    lhsT1[64:128, :] = lhsT1[0:64, :]
    lhsT2 = np.zeros((128, 128), np.float16)
    lhsT2[0:64, 0:64] = w2
    lhsT2[64:128, 64:128] = w2
    lhsT3 = np.zeros((128, 4, 32), np.float16)
    for qq in range(4):
        lhsT3[0:64, qq, 8 * qq:8 * qq + 3] = w3
        lhsT3[64:128, qq, 8 * qq + 3:8 * qq + 6] = w3
    b2rep = np.concatenate([b2, b2]).astype(np.float32).reshape(128, 1)
    b3rep = np.zeros((32, 1), np.float32)
    for qq in range(4):
        b3rep[8 * qq:8 * qq + 3, 0] = b3
        b3rep[8 * qq + 3:8 * qq + 6, 0] = b3
    return lhsT1.reshape(128, 128), lhsT2, lhsT3.reshape(128, 128), b2rep, b3rep


def _patch_table(emb):
    e = np.asarray(emb, dtype=np.float32).reshape(RY, RX, F)
    xs = np.arange(RX)
    x1 = np.minimum(xs + 1, RX - 1)
    ys = np.arange(RY)
    y1 = np.minimum(ys + 1, RY - 1)
    pt = np.zeros((RY, RX, PTROW), dtype=np.float16)
    pt[:, :, 0:F] = e
    pt[:, :, F:2 * F] = e[:, x1, :]
    pt[:, :, 2 * F:3 * F] = e[y1, :, :]
    pt[:, :, 3 * F:4 * F] = e[y1][:, x1, :]
    return np.ascontiguousarray(pt.reshape(RX * RY, PTROW))


def _out_maps(B):
    """(rank_local, feat, valid) lookup arrays for the [32, 1024] chunk tile.

    Row 8qq+r (r<6), col c: h=c//512, m=c%512, b=m//128, p=m%128;
    point col t = 16qq + 4b + 2h + r//3; rank = t*128 + p; feat = r%3.
    """
    rows = np.arange(32)[:, None]
    cols = np.arange(2 * QCOLS)[None, :]
    qq = rows // 8
    r = rows % 8
    h = cols // QCOLS
    m = cols % QCOLS
    b = m // 128
    p = m % 128
    t = 16 * qq + 4 * b + 2 * h + (r // 3)
    rank = t * 128 + p
    feat = np.broadcast_to(r % 3, rank.shape)
    valid = np.broadcast_to(r < 6, rank.shape)
    return rank, feat, valid


def _prep_in_maps(x, emb, w1, b1, w2, b2, w3, b3, n_cores):
    x = np.asarray(x, np.float32)
    N = x.shape[0]
    B = N // n_cores
    T = B // P
    n_chunks = B // CHPTS
    cell, c4 = _host_coefs(x)
    order = np.argsort(cell, kind="stable")
    cell_s = cell[order]
    xs_idf = x[order, 0].astype(np.float16)
    c4_s = c4[order]
    pt = _patch_table(emb)
    lhsT1, lhsT2, lhsT3, b2rep, b3rep = _host_prep_weights(w1, b1, w2, b2, w3, b3)
    in_maps = []
    for k in range(n_cores):
        ci = cell_s[k * B:(k + 1) * B]
        ptw = np.empty((n_chunks * WIN, PTROW), np.float16)
        idx16 = np.empty((16, n_chunks * CHPTS // 16), np.int16)
        for c in range(n_chunks):
            cc = ci[c * CHPTS:(c + 1) * CHPTS]
            base = int(np.clip((int(cc[0]) + int(cc[-1]) + 1) // 2 - WIN // 2,
                               0, RX * RY - WIN))
            lo = cc - base
            assert lo.min() >= 0 and lo.max() < WIN, (
                f"window miss core {k} chunk {c}: {lo.min()} {lo.max()}")
            ptw[c * WIN:(c + 1) * WIN] = pt[base:base + WIN]
            idx16[:, c * (CHPTS // 16):(c + 1) * (CHPTS // 16)] = (
                lo.astype(np.int16).reshape(CHPTS // 16, 16).T)
        # rank r = t*128 + p -> [p, t] column-major partition layout
        c4pm = np.ascontiguousarray(
            c4_s[k * B:(k + 1) * B].reshape(T, P, 4).transpose(1, 0, 2)
        ).reshape(P, T * 4)
        idfpm = np.ascontiguousarray(
            xs_idf[k * B:(k + 1) * B].reshape(T, P).T)
        in_maps.append({
            "c4pm": c4pm,
            "idfpm": idfpm,
            "ptw": ptw,
            "idx16": idx16,
            "lhsT1": lhsT1,
            "lhsT2": lhsT2,
            "lhsT3": lhsT3,
            "ident": np.eye(128, dtype=np.float16),
            "b2rep": b2rep,
            "b3rep": b3rep,
        })
    return in_maps, order


_CACHE = {}


def kernel(x, emb, w1, b1, w2, b2, w3, b3):
    from concourse.bass_utils import run_bass_kernel_spmd

    x = np.asarray(x, np.float32)
    N = x.shape[0]
    B = N // N_CORES
    n_chunks = B // CHPTS

    in_maps, order = _prep_in_maps(x, emb, w1, b1, w2, b2, w3, b3,
                                   n_cores=N_CORES)

    key = (B,)
    if key not in _CACHE:
        nc_new = _build_bass(B)
        nc_new.compile()
        _CACHE[key] = nc_new
    nc = _CACHE[key]

    trace = os.environ.get("KERNEL_TRACE", "0") == "1"
    res = run_bass_kernel_spmd(
        nc, in_maps, core_ids=list(range(N_CORES)), trace=trace
    )
    if trace and res.exec_time_ns is not None:
        print(f"HW exec time: {res.exec_time_ns} ns")

    rank_l, feat, valid = _out_maps(B)
    rows, cols = np.nonzero(valid)
    rk = rank_l[rows, cols]
    ft = feat[rows, cols]
    y_sorted = np.empty((N, 3), np.float32)
    for k in range(N_CORES):
        yT = np.asarray(res.results[k]["yT"], np.float32)
        yc = yT.reshape(32, n_chunks, 2 * QCOLS).transpose(1, 0, 2)
        base = k * B + np.arange(n_chunks)[:, None] * CHPTS
        y_sorted[base + rk[None, :], np.broadcast_to(ft, (n_chunks, ft.size))] = (
            yc[:, rows, cols])
    y = np.empty((N, 3), np.float32)
    y[order, :] = y_sorted
    return y
